# revision 17
# baseline (speedup 1.0000x reference)
"""Trainium2 Bass kernel for nn_DenseSparsePreEmbedding.

Math refactoring (verified bit-exact vs the jax reference on CPU):
    fixed_emb @ W_fixed  == (fixed_table @ W_fixed)[fixed_features]
    sparse_emb @ W_sparse== (concat(tabs) @ W_sparse)[cv]  with cv the
                            combined per-token sparse code (last write wins,
                            sentinel 256 -> zero row for untouched tokens)
so the whole module collapses to a dual embedding gather + add:
    out[n] = tabA[ffn] + tabB[cvn]
with tabA = fixed_table @ W_fixed + b   [2048, 128] f32
     tabB = concat(tab0..3) @ W_sparse (+ zero row)  [257, 128] f32

Device kernel (SPMD over 8 cores, 125000 tokens each):
  - int16 index arrays (dma_gather wrapped layout) preloaded to SBUF
  - per 1024-token tile: two gpsimd.dma_gather (512B rows from HBM) spread
    over 4 SWDGE queues (parallel Q7 descriptor gen), DVE add, HWDGE store.
Measured: 1.584 ms NEFF exec across 8 cores, bit-exact vs the reference.
Hard limits learned on HW: one dma_gather must keep ceil(num_idxs/128)*8+1
descriptors per SDMA lane under ~127 (num_idxs <= 1920), else the device
wedges; num_swdge_queues=4 lets gathers on different queues run on
different Q7 core pairs concurrently.
"""

import numpy as np

N = 1_000_000
NCORES = 8
PER = N // NCORES          # 125000 tokens per core
V = 2048
D = 128
NSPARSE = 257              # 4*64 sparse rows + zero sentinel row
import os as _os

TT = int(_os.environ.get("KTT", "1024"))    # tokens per tile (ring limit: <=1920 idx/op)
NQUEUES = int(_os.environ.get("KNQ", "4"))  # SWDGE queues to spread gathers over
SCRATCH = None             # dynamic_dma_scratch_size override (None = default 16KB)
BUFS = int(_os.environ.get("KBUFS", "6"))   # work tile-pool buffers
ILV = int(_os.environ.get("KILV", "0"))     # interleaved single-gather mode
NTAB = V + NSPARSE                          # 2305 combined table rows (A' ++ Btilde)
PAD = 125056               # per-core tokens padded (tile grid, mult of 2048)
COLS = PAD // 16           # 7816 wrapped-index columns

_cache = {}


def _build_nc(per_core=PER, tt=TT, nqueues=NQUEUES, scratch=SCRATCH, bufs=3):
    import concourse.bacc as bacc
    import concourse.mybir as mybir
    import concourse.tile as tile

    nfull = per_core // tt
    tailv = per_core - nfull * tt
    tailp = ((tailv + 127) // 128) * 128
    pad = nfull * tt + tailp
    cols = pad // 16

    kw = {} if scratch is None else {"dynamic_dma_scratch_size": scratch}
    if nqueues > 1:
        kw["num_swdge_queues"] = nqueues
    nc = bacc.Bacc(
        "TRN2",
        target_bir_lowering=False,
        debug=False,
        enable_asserts=False,
        **kw,
    )
    idxa_t = nc.dram_tensor("idxa", [128, cols], mybir.dt.int16, kind="ExternalInput")
    idxb_t = nc.dram_tensor("idxb", [128, cols], mybir.dt.int16, kind="ExternalInput")
    taba_t = nc.dram_tensor("taba", [V, D], mybir.dt.float32, kind="ExternalInput")
    tabb_t = nc.dram_tensor("tabb", [NSPARSE, D], mybir.dt.float32, kind="ExternalInput")
    out_t = nc.dram_tensor("out", [per_core, D], mybir.dt.float32, kind="ExternalOutput")

    idxa = idxa_t.ap()
    idxb = idxb_t.ap()
    taba = taba_t.ap()
    tabb = tabb_t.ap()
    out = out_t.ap()

    with tile.TileContext(nc) as tc:
        with (
            tc.tile_pool(name="idxp", bufs=1) as ip,
            tc.tile_pool(name="work", bufs=bufs) as wp,
        ):
            ia = ip.tile([128, cols], mybir.dt.int16, tag="ia")
            ib = ip.tile([128, cols], mybir.dt.int16, tag="ib")
            nc.sync.dma_start(out=ia[:], in_=idxa)
            nc.sync.dma_start(out=ib[:], in_=idxb)

            ntiles = nfull + (1 if tailp else 0)
            for t in range(ntiles):
                ni = tt if t < nfull else tailp      # gathered (padded) tokens
                valid = tt if t < nfull else tailv   # rows actually stored
                nblk = (ni + 127) // 128
                c0 = (t * tt) // 16
                da = wp.tile([128, nblk, 128], mybir.dt.float32, tag="da")
                db = wp.tile([128, nblk, 128], mybir.dt.float32, tag="db")
                qa = (2 * t) % nqueues if nqueues > 1 else 0
                qb = (2 * t + 1) % nqueues if nqueues > 1 else 0
                nc.gpsimd.dma_gather(
                    da[:], taba, ia[:, c0 : c0 + ni // 16], ni, ni, D, queue_num=qa
                )
                nc.gpsimd.dma_gather(
                    db[:], tabb, ib[:, c0 : c0 + ni // 16], ni, ni, D, queue_num=qb
                )
                nc.vector.tensor_add(out=da[:], in0=da[:], in1=db[:])
                r0 = t * tt
                fb = valid // 128
                rem = valid - fb * 128
                if fb:
                    ov = out[r0 : r0 + fb * 128, :].rearrange(
                        "(b p) e -> p b e", p=128
                    )
                    nc.sync.dma_start(out=ov, in_=da[:, :fb, :])
                if rem:
                    ov2 = out[r0 + fb * 128 : r0 + valid, :].rearrange(
                        "(b p) e -> p b e", p=rem
                    )
                    nc.sync.dma_start(out=ov2, in_=da[:rem, fb : fb + 1, :])
    nc.compile()
    return nc


def _build_nc_ilv(per_core=PER, ttok=896, nqueues=NQUEUES, scratch=SCRATCH, bufs=BUFS):
    """Interleaved mode: one dma_gather per tile from the combined table.
    Index stream per 128-token chunk: [ff(128), cv+2048(128)], so gathered
    blocks alternate A/B on the same partitions; DVE adds block-pairs."""
    import concourse.bacc as bacc
    import concourse.mybir as mybir
    import concourse.tile as tile

    assert ttok % 128 == 0
    nfull = per_core // ttok
    tailv = per_core - nfull * ttok          # valid tail tokens
    tailp = ((tailv + 127) // 128) * 128     # padded tail tokens
    pad = nfull * ttok + tailp
    nidx = 2 * pad                           # interleaved index count
    cols = nidx // 16

    kw = {} if scratch is None else {"dynamic_dma_scratch_size": scratch}
    if nqueues > 1:
        kw["num_swdge_queues"] = nqueues
    nc = bacc.Bacc(
        "TRN2", target_bir_lowering=False, debug=False, enable_asserts=False, **kw
    )
    idx_t = nc.dram_tensor("idxab", [128, cols], mybir.dt.int16, kind="ExternalInput")
    tab_t = nc.dram_tensor("tabab", [NTAB, D], mybir.dt.float32, kind="ExternalInput")
    out_t = nc.dram_tensor("out", [per_core, D], mybir.dt.float32, kind="ExternalOutput")
    idx = idx_t.ap()
    tab = tab_t.ap()
    out = out_t.ap()

    with tile.TileContext(nc) as tc:
        with (
            tc.tile_pool(name="idxp", bufs=1) as ip,
            tc.tile_pool(name="work", bufs=bufs) as wp,
        ):
            isb = ip.tile([128, cols], mybir.dt.int16, tag="i")
            nc.sync.dma_start(out=isb[:], in_=idx)
            ntiles = nfull + (1 if tailp else 0)
            op = 0
            for t in range(ntiles):
                tok = ttok if t < nfull else tailp
                ni = 2 * tok
                npair = tok // 128
                c0 = (2 * ttok // 16) * t
                q = op % nqueues if nqueues > 1 else 0
                op += 1
                g4 = wp.tile([128, npair, 2, 128], mybir.dt.float32, tag="g")
                cmp = wp.tile([128, npair, 128], mybir.dt.float32, tag="c")
                gv = g4[:].rearrange("p a b e -> p (a b) e")
                nc.gpsimd.dma_gather(
                    gv, tab, isb[:, c0 : c0 + ni // 16], ni, ni, D, queue_num=q
                )
                nc.vector.tensor_add(
                    out=cmp[:], in0=g4[:, :, 0, :], in1=g4[:, :, 1, :]
                )
                r0 = t * ttok
                valid = tok if t < nfull else tailv
                fb = valid // 128
                rem = valid - fb * 128
                if fb:
                    ov = out[r0 : r0 + fb * 128, :].rearrange(
                        "(b p) e -> p b e", p=128
                    )
                    nc.sync.dma_start(out=ov, in_=cmp[:, :fb, :])
                if rem:
                    ov2 = out[r0 + fb * 128 : r0 + valid, :].rearrange(
                        "(b p) e -> p b e", p=rem
                    )
                    nc.sync.dma_start(out=ov2, in_=cmp[:rem, fb : fb + 1, :])
    nc.compile()
    return nc


def _get_nc():
    if "nc" not in _cache:
        if ILV:
            _cache["nc"] = _build_nc_ilv(
                per_core=PER, ttok=TT, nqueues=NQUEUES, scratch=SCRATCH, bufs=BUFS
            )
        else:
            _cache["nc"] = _build_nc(
                per_core=PER, tt=TT, nqueues=NQUEUES, scratch=SCRATCH, bufs=BUFS
            )
    return _cache["nc"]


def _wrap_idx(arr_i16):
    """[PAD] int16 -> [128, COLS] dma_gather wrapped layout: index i lives at
    [i % 16, i // 16]; the 16-row block is replicated to fill 128 partitions."""
    w16 = arr_i16.reshape(-1, 16).T  # [16, COLS]
    return np.ascontiguousarray(np.tile(w16, (8, 1)))  # [128, COLS]


def kernel(
    fixed_features,
    idx0, val0, idx1, val1, idx2, val2, idx3, val3,
    fixed_table, tab0, tab1, tab2, tab3, W_fixed, W_sparse, b,
):
    from concourse.bass_utils import run_bass_kernel_spmd

    ff = np.asarray(fixed_features)
    # combined sparse code per token; 256 = untouched sentinel (zero row).
    cv = np.full(N, 256, dtype=np.int32)
    for k, (ii, vv) in enumerate(
        ((idx0, val0), (idx1, val1), (idx2, val2), (idx3, val3))
    ):
        cv[np.asarray(ii)] = k * 64 + np.asarray(vv).astype(np.int32)

    ft = np.asarray(fixed_table, dtype=np.float32)
    wf = np.asarray(W_fixed, dtype=np.float32)
    ws = np.asarray(W_sparse, dtype=np.float32)
    bb = np.asarray(b, dtype=np.float32)
    taba = (ft @ wf + bb).astype(np.float32)
    tabs = np.concatenate(
        [np.asarray(t, dtype=np.float32) for t in (tab0, tab1, tab2, tab3)], axis=0
    )
    tabb = np.concatenate([tabs @ ws, np.zeros((1, D), np.float32)], axis=0)
    tabb = np.ascontiguousarray(tabb.astype(np.float32))

    nfull = PER // TT
    tailp = ((PER - nfull * TT + 127) // 128) * 128
    padt = nfull * TT + tailp
    if ILV:
        tabab = np.ascontiguousarray(np.concatenate([taba, tabb], axis=0))
    in_maps = []
    for c in range(NCORES):
        sl = slice(c * PER, (c + 1) * PER)
        if ILV:
            fa = np.zeros(padt, dtype=np.int16)
            fa[:PER] = ff[sl].astype(np.int16)
            fbv = np.full(padt, 256 + 2048, dtype=np.int16)
            fbv[:PER] = cv[sl].astype(np.int16) + 2048
            seq = np.stack(
                [fa.reshape(-1, 128), fbv.reshape(-1, 128)], axis=1
            ).reshape(-1)
            in_maps.append({"idxab": _wrap_idx(seq), "tabab": tabab})
            continue
        fa = np.zeros(padt, dtype=np.int16)
        fa[:PER] = ff[sl].astype(np.int16)
        fbv = np.full(padt, 256, dtype=np.int16)
        fbv[:PER] = cv[sl].astype(np.int16)
        in_maps.append(
            {
                "idxa": _wrap_idx(fa),
                "idxb": _wrap_idx(fbv),
                "taba": taba,
                "tabb": tabb,
            }
        )

    nc = _get_nc()
    res = run_bass_kernel_spmd(nc, in_maps, core_ids=list(range(NCORES)))
    _cache["last_results"] = res
    out = np.concatenate([res.results[c]["out"] for c in range(NCORES)], axis=0)
    return out


# revision 19
# speedup vs baseline: 1.2302x; 1.2302x over previous
"""Trainium2 Bass kernel for nn_DenseSparsePreEmbedding.

Math refactoring (verified bit-exact vs the jax reference on CPU):
    fixed_emb @ W_fixed  == (fixed_table @ W_fixed)[fixed_features]
    sparse_emb @ W_sparse== (concat(tabs) @ W_sparse)[cv]  with cv the
                            combined per-token sparse code (last write wins,
                            sentinel 256 -> zero row for untouched tokens)
so the whole module collapses to a dual embedding gather + add:
    out[n] = tabA[ffn] + tabB[cvn]
with tabA = fixed_table @ W_fixed + b   [2048, 128] f32
     tabB = concat(tab0..3) @ W_sparse (+ zero row)  [257, 128] f32

Device kernel (SPMD over 8 cores, 125000 tokens each):
  - int16 index arrays (dma_gather wrapped layout) preloaded to SBUF
  - per 1024-token tile: two gpsimd.dma_gather (512B rows from HBM) spread
    over 4 SWDGE queues (parallel Q7 descriptor gen), DVE add, HWDGE store.
Measured: 1.584 ms NEFF exec across 8 cores, bit-exact vs the reference.
Hard limits learned on HW: one dma_gather must keep ceil(num_idxs/128)*8+1
descriptors per SDMA lane under ~127 (num_idxs <= 1920), else the device
wedges; num_swdge_queues=4 lets gathers on different queues run on
different Q7 core pairs concurrently.
"""

import numpy as np

N = 1_000_000
NCORES = 8
PER = N // NCORES          # 125000 tokens per core
V = 2048
D = 128
NSPARSE = 257              # 4*64 sparse rows + zero sentinel row
import os as _os

TT = int(_os.environ.get("KTT", "1024"))    # tokens per tile (ring limit: <=1920 idx/op)
NQUEUES = int(_os.environ.get("KNQ", "4"))  # SWDGE queues to spread gathers over
SCRATCH = None             # dynamic_dma_scratch_size override (None = default 16KB)
BUFS = int(_os.environ.get("KBUFS", "6"))   # work tile-pool buffers
ILV = int(_os.environ.get("KILV", "0"))     # interleaved single-gather mode
SP = bool(int(_os.environ.get("KSP", "1"))) # dma_gather single_packet flag
NTAB = V + NSPARSE                          # 2305 combined table rows (A' ++ Btilde)
PAD = 125056               # per-core tokens padded (tile grid, mult of 2048)
COLS = PAD // 16           # 7816 wrapped-index columns

_cache = {}


def _build_nc(per_core=PER, tt=TT, nqueues=NQUEUES, scratch=SCRATCH, bufs=3):
    import concourse.bacc as bacc
    import concourse.mybir as mybir
    import concourse.tile as tile

    nfull = per_core // tt
    tailv = per_core - nfull * tt
    tailp = ((tailv + 127) // 128) * 128
    pad = nfull * tt + tailp
    cols = pad // 16

    kw = {} if scratch is None else {"dynamic_dma_scratch_size": scratch}
    if nqueues > 1:
        kw["num_swdge_queues"] = nqueues
    nc = bacc.Bacc(
        "TRN2",
        target_bir_lowering=False,
        debug=False,
        enable_asserts=False,
        **kw,
    )
    idxa_t = nc.dram_tensor("idxa", [128, cols], mybir.dt.int16, kind="ExternalInput")
    idxb_t = nc.dram_tensor("idxb", [128, cols], mybir.dt.int16, kind="ExternalInput")
    taba_t = nc.dram_tensor("taba", [V, D], mybir.dt.float32, kind="ExternalInput")
    tabb_t = nc.dram_tensor("tabb", [NSPARSE, D], mybir.dt.float32, kind="ExternalInput")
    out_t = nc.dram_tensor("out", [per_core, D], mybir.dt.float32, kind="ExternalOutput")

    idxa = idxa_t.ap()
    idxb = idxb_t.ap()
    taba = taba_t.ap()
    tabb = tabb_t.ap()
    out = out_t.ap()

    with tile.TileContext(nc) as tc:
        with (
            tc.tile_pool(name="idxp", bufs=1) as ip,
            tc.tile_pool(name="work", bufs=bufs) as wp,
        ):
            ia = ip.tile([128, cols], mybir.dt.int16, tag="ia")
            ib = ip.tile([128, cols], mybir.dt.int16, tag="ib")
            nc.sync.dma_start(out=ia[:], in_=idxa)
            nc.sync.dma_start(out=ib[:], in_=idxb)

            ntiles = nfull + (1 if tailp else 0)
            for t in range(ntiles):
                ni = tt if t < nfull else tailp      # gathered (padded) tokens
                valid = tt if t < nfull else tailv   # rows actually stored
                nblk = (ni + 127) // 128
                c0 = (t * tt) // 16
                da = wp.tile([128, nblk, 128], mybir.dt.float32, tag="da")
                db = wp.tile([128, nblk, 128], mybir.dt.float32, tag="db")
                qa = (2 * t) % nqueues if nqueues > 1 else 0
                qb = (2 * t + 1) % nqueues if nqueues > 1 else 0
                nc.gpsimd.dma_gather(
                    da[:], taba, ia[:, c0 : c0 + ni // 16], ni, ni, D,
                    queue_num=qa, single_packet=SP,
                )
                nc.gpsimd.dma_gather(
                    db[:], tabb, ib[:, c0 : c0 + ni // 16], ni, ni, D,
                    queue_num=qb, single_packet=SP,
                )
                nc.vector.tensor_add(out=da[:], in0=da[:], in1=db[:])
                r0 = t * tt
                fb = valid // 128
                rem = valid - fb * 128
                if fb:
                    ov = out[r0 : r0 + fb * 128, :].rearrange(
                        "(b p) e -> p b e", p=128
                    )
                    nc.sync.dma_start(out=ov, in_=da[:, :fb, :])
                if rem:
                    ov2 = out[r0 + fb * 128 : r0 + valid, :].rearrange(
                        "(b p) e -> p b e", p=rem
                    )
                    nc.sync.dma_start(out=ov2, in_=da[:rem, fb : fb + 1, :])
    nc.compile()
    return nc


def _build_nc_ilv(per_core=PER, ttok=896, nqueues=NQUEUES, scratch=SCRATCH, bufs=BUFS):
    """Interleaved mode: one dma_gather per tile from the combined table.
    Index stream per 128-token chunk: [ff(128), cv+2048(128)], so gathered
    blocks alternate A/B on the same partitions; DVE adds block-pairs."""
    import concourse.bacc as bacc
    import concourse.mybir as mybir
    import concourse.tile as tile

    assert ttok % 128 == 0
    nfull = per_core // ttok
    tailv = per_core - nfull * ttok          # valid tail tokens
    tailp = ((tailv + 127) // 128) * 128     # padded tail tokens
    pad = nfull * ttok + tailp
    nidx = 2 * pad                           # interleaved index count
    cols = nidx // 16

    kw = {} if scratch is None else {"dynamic_dma_scratch_size": scratch}
    if nqueues > 1:
        kw["num_swdge_queues"] = nqueues
    nc = bacc.Bacc(
        "TRN2", target_bir_lowering=False, debug=False, enable_asserts=False, **kw
    )
    idx_t = nc.dram_tensor("idxab", [128, cols], mybir.dt.int16, kind="ExternalInput")
    tab_t = nc.dram_tensor("tabab", [NTAB, D], mybir.dt.float32, kind="ExternalInput")
    out_t = nc.dram_tensor("out", [per_core, D], mybir.dt.float32, kind="ExternalOutput")
    idx = idx_t.ap()
    tab = tab_t.ap()
    out = out_t.ap()

    with tile.TileContext(nc) as tc:
        with (
            tc.tile_pool(name="idxp", bufs=1) as ip,
            tc.tile_pool(name="work", bufs=bufs) as wp,
        ):
            isb = ip.tile([128, cols], mybir.dt.int16, tag="i")
            nc.sync.dma_start(out=isb[:], in_=idx)
            ntiles = nfull + (1 if tailp else 0)
            op = 0
            for t in range(ntiles):
                tok = ttok if t < nfull else tailp
                ni = 2 * tok
                npair = tok // 128
                c0 = (2 * ttok // 16) * t
                q = op % nqueues if nqueues > 1 else 0
                op += 1
                g4 = wp.tile([128, npair, 2, 128], mybir.dt.float32, tag="g")
                cmp = wp.tile([128, npair, 128], mybir.dt.float32, tag="c")
                gv = g4[:].rearrange("p a b e -> p (a b) e")
                nc.gpsimd.dma_gather(
                    gv, tab, isb[:, c0 : c0 + ni // 16], ni, ni, D, queue_num=q
                )
                nc.vector.tensor_add(
                    out=cmp[:], in0=g4[:, :, 0, :], in1=g4[:, :, 1, :]
                )
                r0 = t * ttok
                valid = tok if t < nfull else tailv
                fb = valid // 128
                rem = valid - fb * 128
                if fb:
                    ov = out[r0 : r0 + fb * 128, :].rearrange(
                        "(b p) e -> p b e", p=128
                    )
                    nc.sync.dma_start(out=ov, in_=cmp[:, :fb, :])
                if rem:
                    ov2 = out[r0 + fb * 128 : r0 + valid, :].rearrange(
                        "(b p) e -> p b e", p=rem
                    )
                    nc.sync.dma_start(out=ov2, in_=cmp[:rem, fb : fb + 1, :])
    nc.compile()
    return nc


def _get_nc():
    if "nc" not in _cache:
        if ILV:
            _cache["nc"] = _build_nc_ilv(
                per_core=PER, ttok=TT, nqueues=NQUEUES, scratch=SCRATCH, bufs=BUFS
            )
        else:
            _cache["nc"] = _build_nc(
                per_core=PER, tt=TT, nqueues=NQUEUES, scratch=SCRATCH, bufs=BUFS
            )
    return _cache["nc"]


def _wrap_idx(arr_i16):
    """[PAD] int16 -> [128, COLS] dma_gather wrapped layout: index i lives at
    [i % 16, i // 16]; the 16-row block is replicated to fill 128 partitions."""
    w16 = arr_i16.reshape(-1, 16).T  # [16, COLS]
    return np.ascontiguousarray(np.tile(w16, (8, 1)))  # [128, COLS]


def kernel(
    fixed_features,
    idx0, val0, idx1, val1, idx2, val2, idx3, val3,
    fixed_table, tab0, tab1, tab2, tab3, W_fixed, W_sparse, b,
):
    from concourse.bass_utils import run_bass_kernel_spmd

    ff = np.asarray(fixed_features)
    # combined sparse code per token; 256 = untouched sentinel (zero row).
    cv = np.full(N, 256, dtype=np.int32)
    for k, (ii, vv) in enumerate(
        ((idx0, val0), (idx1, val1), (idx2, val2), (idx3, val3))
    ):
        cv[np.asarray(ii)] = k * 64 + np.asarray(vv).astype(np.int32)

    ft = np.asarray(fixed_table, dtype=np.float32)
    wf = np.asarray(W_fixed, dtype=np.float32)
    ws = np.asarray(W_sparse, dtype=np.float32)
    bb = np.asarray(b, dtype=np.float32)
    taba = (ft @ wf + bb).astype(np.float32)
    tabs = np.concatenate(
        [np.asarray(t, dtype=np.float32) for t in (tab0, tab1, tab2, tab3)], axis=0
    )
    tabb = np.concatenate([tabs @ ws, np.zeros((1, D), np.float32)], axis=0)
    tabb = np.ascontiguousarray(tabb.astype(np.float32))

    nfull = PER // TT
    tailp = ((PER - nfull * TT + 127) // 128) * 128
    padt = nfull * TT + tailp
    if ILV:
        tabab = np.ascontiguousarray(np.concatenate([taba, tabb], axis=0))
    in_maps = []
    for c in range(NCORES):
        sl = slice(c * PER, (c + 1) * PER)
        if ILV:
            fa = np.zeros(padt, dtype=np.int16)
            fa[:PER] = ff[sl].astype(np.int16)
            fbv = np.full(padt, 256 + 2048, dtype=np.int16)
            fbv[:PER] = cv[sl].astype(np.int16) + 2048
            seq = np.stack(
                [fa.reshape(-1, 128), fbv.reshape(-1, 128)], axis=1
            ).reshape(-1)
            in_maps.append({"idxab": _wrap_idx(seq), "tabab": tabab})
            continue
        fa = np.zeros(padt, dtype=np.int16)
        fa[:PER] = ff[sl].astype(np.int16)
        fbv = np.full(padt, 256, dtype=np.int16)
        fbv[:PER] = cv[sl].astype(np.int16)
        in_maps.append(
            {
                "idxa": _wrap_idx(fa),
                "idxb": _wrap_idx(fbv),
                "taba": taba,
                "tabb": tabb,
            }
        )

    nc = _get_nc()
    res = run_bass_kernel_spmd(nc, in_maps, core_ids=list(range(NCORES)))
    _cache["last_results"] = res
    out = np.concatenate([res.results[c]["out"] for c in range(NCORES)], axis=0)
    return out
